# revision 39
# baseline (speedup 1.0000x reference)
"""CAM-module (channel attention) Trainium2 kernel.

Problem: B=4 samples, C=64, H=W=256 (N=65536 px). concat(rgb,hsv,lab) ->
X [192, N] per sample; q/k/v = 1x1-conv projections (W [64,192] + bias);
energy = q @ k^T * C^-0.5 -> softmax over last dim -> out = att @ v.

Sharding: 8 cores = 4 samples x 2 spatial halves (32768 px each). Each
core computes a partial energy over its half; a 16 KiB pairwise AllReduce
([[0,1],[2,3],[4,5],[6,7]]) completes the C x C energy, then each core
computes out for its own half. HBM traffic is the 32 MiB/core minimum.

Precision: the softmax logits have std ~850, so energy needs ~fp32
accuracy. Projections run as 3-pass fp16 hi/lo splits (X = Xh + Xl,
W = Wh + Wl; passes Xh@Wh + Xh@Wl + Xl@Wh, fp32 PSUM accumulate), which
keeps the dropped term at ~2^-22. The energy matmul itself runs in true
fp32 (4 cyc/row, tiny N). v uses Wh@(Xh+Xl); out = fp16(att) @ (vh+vl)
via a stacked [vh; vl] tile so both halves go through one matmul.
Measured vs fp64: absmax ~2e-3 == the fp32 reference's own envelope.

Biases fold in exactly via an appended ones-row on X (lab chunk becomes
65 partitions) and bias rows on the weight chunks (hi+lo).
"""

import sys
import numpy as np

if '/opt/trn_rl_repo' not in sys.path:
    sys.path.insert(0, '/opt/trn_rl_repo')

B, C, H, W = 4, 64, 256, 256
N = H * W                 # 65536 px per sample
NHALF = N // 2            # 32768 px per core
PX = 2048                 # streaming tile (px)
NIT = NHALF // PX         # 16
SUB = 128                 # qkT subtile (px) = matmul M
NSUB = PX // SUB          # 16
VC = 512                  # v / out chunk (px) = matmul N
NVC = PX // VC            # 4
NCORES = 8

_CACHE = {}


def _build_bass(single_core=False):
    import concourse.bacc as bacc
    import concourse.mybir as mybir
    from concourse import tile

    F32 = mybir.dt.float32
    F16 = mybir.dt.float16
    Exp = mybir.ActivationFunctionType.Exp

    nc = bacc.Bacc("TRN2", target_bir_lowering=False, debug=False,
                   enable_asserts=False,
                   num_devices=1 if single_core else NCORES)

    xr_d = nc.dram_tensor("x_rgb", [64, NHALF], F32, kind="ExternalInput").ap()
    xs_d = nc.dram_tensor("x_hsv", [64, NHALF], F32, kind="ExternalInput").ap()
    xb_d = nc.dram_tensor("x_lab", [64, NHALF], F32, kind="ExternalInput").ap()
    # packed weights: cols [wqkh 0:128 | wqkl 128:256 | wvh 256:320]
    w0_d = nc.dram_tensor("w0", [128, 320], F16, kind="ExternalInput").ap()
    w1_d = nc.dram_tensor("w1", [65, 320], F16, kind="ExternalInput").ap()
    ident_d = nc.dram_tensor("ident", [64, 64], F32, kind="ExternalInput").ap()
    out_d = nc.dram_tensor("out", [64, NHALF], F32, kind="ExternalOutput").ap()

    with tile.TileContext(nc) as tc:
        with tc.tile_pool(name="const", bufs=1) as const, \
             tc.tile_pool(name="stream", bufs=3) as stream, \
             tc.tile_pool(name="qk", bufs=3) as qkpool, \
             tc.tile_pool(name="outp", bufs=4) as outp, \
             tc.tile_pool(name="qkps", bufs=2, space="PSUM") as qkps, \
             tc.tile_pool(name="vps", bufs=2, space="PSUM") as vps, \
             tc.tile_pool(name="eps", bufs=1, space="PSUM") as eps, \
             tc.tile_pool(name="dram", bufs=1, space="DRAM") as dram:

            w0 = const.tile([128, 320], F16)
            w1 = const.tile([65, 320], F16)
            ident = const.tile([64, 64], F32)
            nc.scalar.dma_start(w0[:], w0_d[:])
            nc.scalar.dma_start(w1[:], w1_d[:])
            nc.scalar.dma_start(ident[:], ident_d[:])
            wqkh0, wqkl0, wvh0 = w0[:, 0:128], w0[:, 128:256], w0[:, 256:320]
            wqkh1, wqkl1, wvh1 = w1[:, 0:128], w1[:, 128:256], w1[:, 256:320]

            # preload the ACT Exp table set off the critical path (~2.7us)
            warm = const.tile([1, 1], F32)
            nc.gpsimd.memset(warm[:], 0.0)
            nc.scalar.activation(warm[:], warm[:], Exp)

            vhl = const.tile([128, NHALF], F16)   # [vh; vl] stacked
            ep = eps.tile([64, 64], F32)          # energy accumulator

            for it in range(NIT):
                sl = slice(it * PX, (it + 1) * PX)
                x0_32 = stream.tile([128, PX], F32, tag="x0_32")
                nc.sync.dma_start(x0_32[0:64, :], xr_d[:, sl])
                nc.sync.dma_start(x0_32[64:128, :], xs_d[:, sl])
                x1_32 = stream.tile([64, PX], F32, tag="x1_32")
                nc.sync.dma_start(x1_32[:], xb_d[:, sl])

                x0h = stream.tile([128, PX], F16, tag="x0h")
                nc.scalar.copy(x0h[:], x0_32[:])
                x0l = stream.tile([128, PX], F16, tag="x0l")
                nc.vector.tensor_sub(x0l[:], x0_32[:], x0h[:])
                x1h = stream.tile([65, PX], F16, tag="x1h")
                nc.scalar.copy(x1h[0:64, :], x1_32[:])
                x1l = stream.tile([65, PX], F16, tag="x1l")
                nc.vector.tensor_sub(x1l[0:64, :], x1_32[:], x1h[0:64, :])
                if it < 3:
                    # ones/zeros rows live in the 3 round-robin pool slots;
                    # later iterations reuse them untouched
                    nc.gpsimd.memset(x1h[64:65, :], 1.0)
                    nc.gpsimd.memset(x1l[64:65, :], 0.0)

                for grp in range(NSUB // 4):   # qkT: 4 subtiles per PSUM bank
                    qkp = qkps.tile([128, 512], F32, tag="qkp")
                    for s4 in range(4):
                        sb = grp * 4 + s4
                        ssl = slice(sb * SUB, (sb + 1) * SUB)
                        osl = slice(s4 * 128, (s4 + 1) * 128)
                        nc.tensor.matmul(qkp[:, osl], x0h[:, ssl], wqkh0[:], start=True, stop=False)
                        nc.tensor.matmul(qkp[:, osl], x0h[:, ssl], wqkl0[:], start=False, stop=False)
                        nc.tensor.matmul(qkp[:, osl], x0l[:, ssl], wqkh0[:], start=False, stop=False)
                        nc.tensor.matmul(qkp[:, osl], x1h[:, ssl], wqkh1[:], start=False, stop=False)
                        nc.tensor.matmul(qkp[:, osl], x1h[:, ssl], wqkl1[:], start=False, stop=False)
                        nc.tensor.matmul(qkp[:, osl], x1l[:, ssl], wqkh1[:], start=False, stop=True)
                    qk_sb = qkpool.tile([128, 512], F32, tag="qk_sb")
                    nc.scalar.copy(qk_sb[:], qkp[:])
                    for s4 in range(4):
                        first = (it == 0 and grp == 0 and s4 == 0)
                        last = (it == NIT - 1 and grp == NSUB // 4 - 1 and s4 == 3)
                        nc.tensor.matmul(ep[:], qk_sb[:, s4 * 128:s4 * 128 + 64],
                                         qk_sb[:, s4 * 128 + 64:s4 * 128 + 128],
                                         start=first, stop=last)

                # v for the PREVIOUS tile (1-pass, 2 chunks per PSUM tile):
                # delaying v by one iteration leaves PE ~7us of v-work to
                # chew on while the AllReduce runs after the final E matmul
                def v_block(vit, vx0h, vx1h):
                    for vg in range(NVC // 2):
                        vp = vps.tile([64, 2 * VC], F32, tag="vp")
                        for h in range(2):
                            vc = vg * 2 + h
                            vsl = slice(vc * VC, (vc + 1) * VC)
                            psl = slice(h * VC, (h + 1) * VC)
                            nc.tensor.matmul(vp[:, psl], wvh0[:], vx0h[:, vsl],
                                             start=True, stop=False)
                            nc.tensor.matmul(vp[:, psl], wvh1[:], vx1h[:, vsl],
                                             start=False, stop=True)
                        gsl = slice(vit * PX + vg * 2 * VC,
                                    vit * PX + (vg + 1) * 2 * VC)
                        nc.scalar.copy(vhl[0:64, gsl], vp[:])
                        nc.vector.tensor_sub(vhl[64:128, gsl], vp[:], vhl[0:64, gsl])

                if it > 0:
                    v_block(it - 1, prev_x0h, prev_x1h)
                prev_x0h, prev_x1h = x0h, x1h

            v_block(NIT - 1, prev_x0h, prev_x1h)

            # partial energy -> pairwise AllReduce
            e_sb = const.tile([64, 64], F32)
            nc.scalar.copy(e_sb[:], ep[:])
            bi = dram.tile([64, 64], F32)
            bo = dram.tile([64, 64], F32)
            nc.sync.dma_start(bi[:], e_sb[:])
            if single_core:
                nc.gpsimd.dma_start(bo[:], bi[:])
            else:
                nc.gpsimd.collective_compute(
                    "AllReduce", mybir.AluOpType.add,
                    replica_groups=[[0, 1], [2, 3], [4, 5], [6, 7]],
                    ins=[bi.opt()], outs=[bo.opt()],
                )
            e2 = const.tile([64, 64], F32)
            nc.sync.dma_start(e2[:], bo[:])

            # softmax over free dim, scale C^-0.5 = 0.125 folded into exp
            m = const.tile([64, 1], F32)
            nc.vector.reduce_max(m[:], e2[:], axis=mybir.AxisListType.X)
            mb = const.tile([64, 1], F32)
            nc.vector.tensor_scalar_mul(mb[:], m[:], -0.125)
            attu = const.tile([64, 64], F32)
            s = const.tile([64, 1], F32)
            nc.scalar.activation(attu[:], e2[:], Exp, bias=mb[:], scale=0.125,
                                 accum_out=s[:])
            r = const.tile([64, 1], F32)
            nc.vector.reciprocal(r[:], s[:])
            att = const.tile([64, 64], F32)
            nc.vector.tensor_scalar_mul(att[:], attu[:], r[:])

            # att^T (PE transpose), cast fp16, stacked twice for [vh; vl]
            atp = vps.tile([64, 64], F32, tag="vp")
            nc.tensor.transpose(atp[:], att[:], ident[:])
            attT2 = const.tile([128, 64], F16)
            nc.scalar.copy(attT2[0:64, :], atp[:])
            nc.scalar.copy(attT2[64:128, :], atp[:])

            # out = att @ (vh + vl), per-512px-chunk pipeline: 3 PSUM chunk
            # slots (2 halves of the vp slots + 1 extra bank), copies
            # alternating ACT/DVE, DMA per 2048px
            out_sb = None
            for oc in range(NHALF // VC):
                if oc % 3 < 2:
                    op = vps.tile([64, VC], F32, tag="vp")
                else:
                    op = eps.tile([64, VC], F32, tag="op2")
                nc.tensor.matmul(op[:], attT2[:], vhl[:, oc * VC:(oc + 1) * VC],
                                 start=True, stop=True)
                w4 = oc % 4
                if w4 == 0:
                    out_sb = outp.tile([64, PX], F32, tag="out_sb")
                dst = out_sb[:, w4 * VC:(w4 + 1) * VC]
                if oc % 2 == 0:
                    nc.scalar.copy(dst, op[:])
                else:
                    nc.vector.tensor_copy(dst, op[:])
                if w4 == 3:
                    g = oc // 4
                    nc.sync.dma_start(out_d[:, g * PX:(g + 1) * PX], out_sb[:])

    nc.compile()
    return nc


def _get_nc():
    if 'nc' not in _CACHE:
        _CACHE['nc'] = _build_bass()
    return _CACHE['nc']


def _split16(a):
    h = a.astype(np.float16)
    l = (a - h.astype(np.float32)).astype(np.float16)
    return h, l


def kernel(rgb, hsv, lab, Wq, bq, Wk, bk, Wv, bv):
    from concourse.bass_utils import run_bass_kernel_spmd

    nc = _get_nc()

    rgb = np.asarray(rgb, dtype=np.float32)
    hsv = np.asarray(hsv, dtype=np.float32)
    lab = np.asarray(lab, dtype=np.float32)
    Wq = np.asarray(Wq, dtype=np.float32)
    Wk = np.asarray(Wk, dtype=np.float32)
    Wv = np.asarray(Wv, dtype=np.float32)
    bq = np.asarray(bq, dtype=np.float32)
    bk = np.asarray(bk, dtype=np.float32)
    bv = np.asarray(bv, dtype=np.float32)

    # weight prep: [192ch + ones-row, outs] with bias row, fp16 hi/lo
    wqk = np.concatenate([Wq.T, Wk.T], axis=1)          # [192, 128]
    bqk = np.concatenate([bq, bk])                      # [128]
    wqk_aug = np.vstack([wqk, bqk[None, :]])            # [193, 128]
    wqkh, wqkl = _split16(wqk_aug)
    wv_aug = np.vstack([Wv.T, bv[None, :]])             # [193, 64]
    wvh, _ = _split16(wv_aug)

    shared = {
        "w0": np.ascontiguousarray(
            np.concatenate([wqkh[0:128], wqkl[0:128], wvh[0:128]], axis=1)),
        "w1": np.ascontiguousarray(
            np.concatenate([wqkh[128:193], wqkl[128:193], wvh[128:193]], axis=1)),
        "ident": np.eye(64, dtype=np.float32),
    }

    in_maps = []
    for c in range(NCORES):
        b, half = c // 2, c % 2
        hs = slice(half * (H // 2), (half + 1) * (H // 2))
        in_maps.append({
            "x_rgb": np.ascontiguousarray(rgb[b, :, hs, :].reshape(C, NHALF)),
            "x_hsv": np.ascontiguousarray(hsv[b, :, hs, :].reshape(C, NHALF)),
            "x_lab": np.ascontiguousarray(lab[b, :, hs, :].reshape(C, NHALF)),
            **shared,
        })

    res = run_bass_kernel_spmd(nc, in_maps, core_ids=list(range(NCORES)),
                               **_CACHE.get('run_kwargs', {}))
    _CACHE['last_results'] = res
    _CACHE['last_in_maps'] = in_maps

    out = np.empty((B, C, H, W), dtype=np.float32)
    for c in range(NCORES):
        b, half = c // 2, c % 2
        hs = slice(half * (H // 2), (half + 1) * (H // 2))
        out[b, :, hs, :] = res.results[c]["out"].reshape(C, H // 2, W)
    return out


# revision 41
# speedup vs baseline: 1.4858x; 1.4858x over previous
"""CAM-module (channel attention) Trainium2 kernel.

Problem: B=4 samples, C=64, H=W=256 (N=65536 px). concat(rgb,hsv,lab) ->
X [192, N] per sample; q/k/v = 1x1-conv projections (W [64,192] + bias);
energy = q @ k^T * C^-0.5 -> softmax over last dim -> out = att @ v.

Sharding: 8 cores = 4 samples x 2 spatial halves (32768 px each). Each
core computes a partial energy over its half; a 16 KiB pairwise AllReduce
([[0,1],[2,3],[4,5],[6,7]]) completes the C x C energy, then each core
computes out for its own half. HBM traffic is the 32 MiB/core minimum.

Precision: the softmax logits have std ~850, so energy needs ~fp32
accuracy. Projections run as 3-pass fp16 hi/lo splits (X = Xh + Xl,
W = Wh + Wl; passes Xh@Wh + Xh@Wl + Xl@Wh, fp32 PSUM accumulate), which
keeps the dropped term at ~2^-22. The energy matmul itself runs in true
fp32 (4 cyc/row, tiny N). v uses Wh@(Xh+Xl); out = fp16(att) @ (vh+vl)
via a stacked [vh; vl] tile so both halves go through one matmul.
Measured vs fp64: absmax ~2e-3 == the fp32 reference's own envelope.

Biases fold in exactly via an appended ones-row on X (lab chunk becomes
65 partitions) and bias rows on the weight chunks (hi+lo).
"""

import sys
import numpy as np

if '/opt/trn_rl_repo' not in sys.path:
    sys.path.insert(0, '/opt/trn_rl_repo')

B, C, H, W = 4, 64, 256, 256
N = H * W                 # 65536 px per sample
NHALF = N // 2            # 32768 px per core
PX = 2048                 # streaming tile (px)
NIT = NHALF // PX         # 16
SUB = 128                 # qkT subtile (px) = matmul M
NSUB = PX // SUB          # 16
VC = 512                  # v / out chunk (px) = matmul N
NVC = PX // VC            # 4
NCORES = 8

_CACHE = {}


def _build_bass(single_core=False):
    import concourse.bacc as bacc
    import concourse.mybir as mybir
    from concourse import tile

    F32 = mybir.dt.float32
    F16 = mybir.dt.float16
    Exp = mybir.ActivationFunctionType.Exp

    nc = bacc.Bacc("TRN2", target_bir_lowering=False, debug=False,
                   enable_asserts=False,
                   num_devices=1 if single_core else NCORES)

    xr_d = nc.dram_tensor("x_rgb", [64, NHALF], F32, kind="ExternalInput").ap()
    xs_d = nc.dram_tensor("x_hsv", [64, NHALF], F32, kind="ExternalInput").ap()
    xb_d = nc.dram_tensor("x_lab", [64, NHALF], F32, kind="ExternalInput").ap()
    # packed weights: cols [wqkh 0:128 | wqkl 128:256 | wvh 256:320]
    w0_d = nc.dram_tensor("w0", [128, 320], F16, kind="ExternalInput").ap()
    w1_d = nc.dram_tensor("w1", [65, 320], F16, kind="ExternalInput").ap()
    ident_d = nc.dram_tensor("ident", [64, 64], F32, kind="ExternalInput").ap()
    out_d = nc.dram_tensor("out", [64, NHALF], F32, kind="ExternalOutput").ap()

    with tile.TileContext(nc) as tc:
        with tc.tile_pool(name="const", bufs=1) as const, \
             tc.tile_pool(name="stream", bufs=3) as stream, \
             tc.tile_pool(name="qk", bufs=4) as qkpool, \
             tc.tile_pool(name="outp", bufs=4) as outp, \
             tc.tile_pool(name="qkps", bufs=2, space="PSUM") as qkps, \
             tc.tile_pool(name="vps", bufs=2, space="PSUM") as vps, \
             tc.tile_pool(name="eps", bufs=1, space="PSUM") as eps, \
             tc.tile_pool(name="dram", bufs=1, space="DRAM") as dram:

            w0 = const.tile([128, 320], F16)
            w1 = const.tile([65, 320], F16)
            ident = const.tile([64, 64], F32)
            nc.scalar.dma_start(w0[:], w0_d[:])
            nc.scalar.dma_start(w1[:], w1_d[:])
            nc.scalar.dma_start(ident[:], ident_d[:])
            wqkh0, wqkl0, wvh0 = w0[:, 0:128], w0[:, 128:256], w0[:, 256:320]
            wqkh1, wqkl1, wvh1 = w1[:, 0:128], w1[:, 128:256], w1[:, 256:320]

            # preload the ACT Exp table set off the critical path (~2.7us)
            warm = const.tile([1, 1], F32)
            nc.gpsimd.memset(warm[:], 0.0)
            nc.scalar.activation(warm[:], warm[:], Exp)

            vhl = const.tile([128, NHALF], F16)   # [vh; vl] stacked
            ep = eps.tile([64, 64], F32)          # energy accumulator

            for it in range(NIT):
                sl = slice(it * PX, (it + 1) * PX)
                x0_32 = stream.tile([128, PX], F32, tag="x0_32")
                nc.sync.dma_start(x0_32[0:64, :], xr_d[:, sl])
                nc.sync.dma_start(x0_32[64:128, :], xs_d[:, sl])
                x1_32 = stream.tile([64, PX], F32, tag="x1_32")
                nc.sync.dma_start(x1_32[:], xb_d[:, sl])

                x0h = stream.tile([128, PX], F16, tag="x0h")
                nc.scalar.copy(x0h[:], x0_32[:])
                x0l = stream.tile([128, PX], F16, tag="x0l")
                nc.vector.tensor_sub(x0l[:], x0_32[:], x0h[:])
                x1h = stream.tile([65, PX], F16, tag="x1h")
                nc.scalar.copy(x1h[0:64, :], x1_32[:])
                x1l = stream.tile([65, PX], F16, tag="x1l")
                nc.vector.tensor_sub(x1l[0:64, :], x1_32[:], x1h[0:64, :])
                if it < 3:
                    # ones/zeros rows live in the 3 round-robin pool slots;
                    # later iterations reuse them untouched
                    nc.gpsimd.memset(x1h[64:65, :], 1.0)
                    nc.gpsimd.memset(x1l[64:65, :], 0.0)

                for grp in range(NSUB // 4):   # qkT: 4 subtiles per PSUM bank
                    qkp = qkps.tile([128, 512], F32, tag="qkp")
                    for s4 in range(4):
                        sb = grp * 4 + s4
                        ssl = slice(sb * SUB, (sb + 1) * SUB)
                        osl = slice(s4 * 128, (s4 + 1) * 128)
                        nc.tensor.matmul(qkp[:, osl], x0h[:, ssl], wqkh0[:], start=True, stop=False)
                        nc.tensor.matmul(qkp[:, osl], x0h[:, ssl], wqkl0[:], start=False, stop=False)
                        nc.tensor.matmul(qkp[:, osl], x0l[:, ssl], wqkh0[:], start=False, stop=False)
                        nc.tensor.matmul(qkp[:, osl], x1h[:, ssl], wqkh1[:], start=False, stop=False)
                        nc.tensor.matmul(qkp[:, osl], x1h[:, ssl], wqkl1[:], start=False, stop=False)
                        nc.tensor.matmul(qkp[:, osl], x1l[:, ssl], wqkh1[:], start=False, stop=True)
                    qk_sb = qkpool.tile([128, 512], F32, tag="qk_sb")
                    nc.scalar.copy(qk_sb[:], qkp[:])
                    for s4 in range(4):
                        first = (it == 0 and grp == 0 and s4 == 0)
                        last = (it == NIT - 1 and grp == NSUB // 4 - 1 and s4 == 3)
                        nc.tensor.matmul(ep[:], qk_sb[:, s4 * 128:s4 * 128 + 64],
                                         qk_sb[:, s4 * 128 + 64:s4 * 128 + 128],
                                         start=first, stop=last)

                # v for the PREVIOUS tile (1-pass, 2 chunks per PSUM tile):
                # delaying v by one iteration leaves PE ~7us of v-work to
                # chew on while the AllReduce runs after the final E matmul
                def v_block(vit, vx0h, vx1h):
                    for vg in range(NVC // 2):
                        vp = vps.tile([64, 2 * VC], F32, tag="vp")
                        for h in range(2):
                            vc = vg * 2 + h
                            vsl = slice(vc * VC, (vc + 1) * VC)
                            psl = slice(h * VC, (h + 1) * VC)
                            nc.tensor.matmul(vp[:, psl], wvh0[:], vx0h[:, vsl],
                                             start=True, stop=False)
                            nc.tensor.matmul(vp[:, psl], wvh1[:], vx1h[:, vsl],
                                             start=False, stop=True)
                        gsl = slice(vit * PX + vg * 2 * VC,
                                    vit * PX + (vg + 1) * 2 * VC)
                        nc.scalar.copy(vhl[0:64, gsl], vp[:])
                        nc.vector.tensor_sub(vhl[64:128, gsl], vp[:], vhl[0:64, gsl])

                if it > 1:
                    v_block(it - 2, *pending.pop(0))
                pending = (pending if it > 0 else []) + [(x0h, x1h)]

            v_block(NIT - 2, *pending[0])
            v_block(NIT - 1, *pending[1])

            # partial energy -> pairwise AllReduce
            e_sb = const.tile([64, 64], F32)
            nc.scalar.copy(e_sb[:], ep[:])
            bi = dram.tile([64, 64], F32)
            bo = dram.tile([64, 64], F32)
            nc.sync.dma_start(bi[:], e_sb[:])
            if single_core:
                nc.gpsimd.dma_start(bo[:], bi[:])
            else:
                nc.gpsimd.collective_compute(
                    "AllReduce", mybir.AluOpType.add,
                    replica_groups=[[0, 1], [2, 3], [4, 5], [6, 7]],
                    ins=[bi.opt()], outs=[bo.opt()],
                )
            e2 = const.tile([64, 64], F32)
            nc.sync.dma_start(e2[:], bo[:])

            # softmax over free dim, scale C^-0.5 = 0.125 folded into exp
            m = const.tile([64, 1], F32)
            nc.vector.reduce_max(m[:], e2[:], axis=mybir.AxisListType.X)
            mb = const.tile([64, 1], F32)
            nc.vector.tensor_scalar_mul(mb[:], m[:], -0.125)
            attu = const.tile([64, 64], F32)
            s = const.tile([64, 1], F32)
            nc.scalar.activation(attu[:], e2[:], Exp, bias=mb[:], scale=0.125,
                                 accum_out=s[:])
            r = const.tile([64, 1], F32)
            nc.vector.reciprocal(r[:], s[:])
            att = const.tile([64, 64], F32)
            nc.vector.tensor_scalar_mul(att[:], attu[:], r[:])

            # att^T (PE transpose), cast fp16, stacked twice for [vh; vl]
            atp = vps.tile([64, 64], F32, tag="vp")
            nc.tensor.transpose(atp[:], att[:], ident[:])
            attT2 = const.tile([128, 64], F16)
            nc.scalar.copy(attT2[0:64, :], atp[:])
            nc.scalar.copy(attT2[64:128, :], atp[:])

            # out = att @ (vh + vl), per-512px-chunk pipeline: 3 PSUM chunk
            # slots (2 halves of the vp slots + 1 extra bank), copies
            # alternating ACT/DVE, DMA per 2048px
            out_sb = None
            for oc in range(NHALF // VC):
                if oc % 3 < 2:
                    op = vps.tile([64, VC], F32, tag="vp")
                else:
                    op = eps.tile([64, VC], F32, tag="op2")
                nc.tensor.matmul(op[:], attT2[:], vhl[:, oc * VC:(oc + 1) * VC],
                                 start=True, stop=True)
                w4 = oc % 4
                if w4 == 0:
                    out_sb = outp.tile([64, PX], F32, tag="out_sb")
                dst = out_sb[:, w4 * VC:(w4 + 1) * VC]
                if oc % 2 == 0:
                    nc.scalar.copy(dst, op[:])
                else:
                    nc.vector.tensor_copy(dst, op[:])
                if w4 == 3:
                    g = oc // 4
                    nc.sync.dma_start(out_d[:, g * PX:(g + 1) * PX], out_sb[:])

    nc.compile()
    return nc


def _get_nc():
    if 'nc' not in _CACHE:
        _CACHE['nc'] = _build_bass()
    return _CACHE['nc']


def _split16(a):
    h = a.astype(np.float16)
    l = (a - h.astype(np.float32)).astype(np.float16)
    return h, l


def kernel(rgb, hsv, lab, Wq, bq, Wk, bk, Wv, bv):
    from concourse.bass_utils import run_bass_kernel_spmd

    nc = _get_nc()

    rgb = np.asarray(rgb, dtype=np.float32)
    hsv = np.asarray(hsv, dtype=np.float32)
    lab = np.asarray(lab, dtype=np.float32)
    Wq = np.asarray(Wq, dtype=np.float32)
    Wk = np.asarray(Wk, dtype=np.float32)
    Wv = np.asarray(Wv, dtype=np.float32)
    bq = np.asarray(bq, dtype=np.float32)
    bk = np.asarray(bk, dtype=np.float32)
    bv = np.asarray(bv, dtype=np.float32)

    # weight prep: [192ch + ones-row, outs] with bias row, fp16 hi/lo
    wqk = np.concatenate([Wq.T, Wk.T], axis=1)          # [192, 128]
    bqk = np.concatenate([bq, bk])                      # [128]
    wqk_aug = np.vstack([wqk, bqk[None, :]])            # [193, 128]
    wqkh, wqkl = _split16(wqk_aug)
    wv_aug = np.vstack([Wv.T, bv[None, :]])             # [193, 64]
    wvh, _ = _split16(wv_aug)

    shared = {
        "w0": np.ascontiguousarray(
            np.concatenate([wqkh[0:128], wqkl[0:128], wvh[0:128]], axis=1)),
        "w1": np.ascontiguousarray(
            np.concatenate([wqkh[128:193], wqkl[128:193], wvh[128:193]], axis=1)),
        "ident": np.eye(64, dtype=np.float32),
    }

    in_maps = []
    for c in range(NCORES):
        b, half = c // 2, c % 2
        hs = slice(half * (H // 2), (half + 1) * (H // 2))
        in_maps.append({
            "x_rgb": np.ascontiguousarray(rgb[b, :, hs, :].reshape(C, NHALF)),
            "x_hsv": np.ascontiguousarray(hsv[b, :, hs, :].reshape(C, NHALF)),
            "x_lab": np.ascontiguousarray(lab[b, :, hs, :].reshape(C, NHALF)),
            **shared,
        })

    res = run_bass_kernel_spmd(nc, in_maps, core_ids=list(range(NCORES)),
                               **_CACHE.get('run_kwargs', {}))
    _CACHE['last_results'] = res
    _CACHE['last_in_maps'] = in_maps

    out = np.empty((B, C, H, W), dtype=np.float32)
    for c in range(NCORES):
        b, half = c // 2, c % 2
        hs = slice(half * (H // 2), (half + 1) * (H // 2))
        out[b, :, hs, :] = res.results[c]["out"].reshape(C, H // 2, W)
    return out
